# revision 5
# baseline (speedup 1.0000x reference)
"""CRF-RNN layer (nn_CRF_RNN_Layer) as a Bass/Tile kernel on 8 trn2 NeuronCores.

Math (reference):
    N = 96*96 pixels, C = 21 classes, 5 mean-field iterations.
    k_spatial / k_bilateral are [N, N] Gaussian kernels; per iteration:
        p = softmax(q); S = Ks @ p; Bi = Kb @ p
        pairwise = (S * ws + Bi * wb) @ C.T;  q = u - pairwise

Device strategy:
    - Row-shard outputs over 8 cores (1152 rows each).
    - Channels padded 21 -> 32; pad logits are -1e30 so softmax pads are 0.
    - Per-channel weights + compat fold on host: CsF[c,k] = ws[c]*C[k,c].
    - Big matmuls in out.T form: S.T[c, i] = sum_j p[j, c] * K[j, i],
      with lhsT = p tiles [128, 32] bf16 and rhs = K tiles [128, 1152] bf16.
    - Kb (data-dependent) is built on device: X = Faug_j . Gaug_i via a
      7-contraction matmul, exp on the scalar engine, stored bf16 (mostly
      SBUF-resident, remainder spilled to HBM).
    - Ks depends only on pixel positions -> host constant, streamed bf16.
    - Per-iteration AllGather of the new logits (147KB/rank) in a
      partition-major block layout so every DMA runs are contiguous >= 1KB.
"""

import numpy as np
import ml_dtypes

from concourse import bacc, mybir, tile
from concourse.bass_utils import run_bass_kernel_spmd

H, W, C = 96, 96, 21
THETA_ALPHA, THETA_BETA, THETA_GAMMA = 8.0, 0.125, 3.0
NITER = 5
NCORES = 8
N = H * W                     # 9216
BAND = N // NCORES            # 1152 rows per core
CP = 32                       # padded channels
TJ = N // 128                 # 72 j-tiles
TB = BAND // 128              # 9 band tiles
RES = 52                      # j-tiles of Kb kept SBUF-resident; rest spilled
NSPILL = TJ - RES
GRP = 4                       # j-tiles per streaming DMA (1.18 MB)
NEG = -1.0e30
CHUNKS = [(0, 512), (512, 512), (1024, 128)]   # psum-bank-aligned N splits of 1152

_CACHE = {}


def _build_nc():
    nc = bacc.Bacc("TRN2", target_bir_lowering=False, debug=False, num_devices=NCORES)
    f32 = mybir.dt.float32
    bf16 = mybir.dt.bfloat16

    at_d = nc.declare_dram_parameter("at", [7, N], f32, isOutput=False)
    bt_d = nc.declare_dram_parameter("bt", [7, BAND], f32, isOutput=False)
    kst_d = nc.declare_dram_parameter("kst", [N, BAND], bf16, isOutput=False)
    uarr_d = nc.declare_dram_parameter("uarr", [128, TJ * CP], f32, isOutput=False)
    uband_d = nc.declare_dram_parameter("uband", [128, TB * CP], f32, isOutput=False)
    csf_d = nc.declare_dram_parameter("csf", [CP, CP], f32, isOutput=False)
    cbf_d = nc.declare_dram_parameter("cbf", [CP, CP], f32, isOutput=False)
    out_d = nc.declare_dram_parameter("out", [128, TB * CP], f32, isOutput=True)

    with tile.TileContext(nc) as tc:
        with (
            tc.tile_pool(name="kres", bufs=1) as kres_pool,
            tc.tile_pool(name="stream", bufs=3) as stream_pool,
            tc.tile_pool(name="state", bufs=1) as state,
            tc.tile_pool(name="small", bufs=2) as small,
            tc.tile_pool(name="dram", bufs=1, space="DRAM") as dram,
        ):
            # ---- constants ----
            csf = state.tile([CP, CP], f32, tag="csf")
            cbf = state.tile([CP, CP], f32, tag="cbf")
            u_band = state.tile([128, TB * CP], f32, tag="uband")
            bt = state.tile([7, BAND], f32, tag="bt")
            nc.sync.dma_start(csf[:], csf_d[:])
            nc.sync.dma_start(cbf[:], cbf_d[:])
            nc.sync.dma_start(u_band[:], uband_d[:])
            nc.sync.dma_start(bt[:], bt_d[:])

            kb_res = kres_pool.tile([128, RES * BAND], bf16, tag="kbres")
            kb_spill = dram.tile([NSPILL * 128, BAND], bf16)

            # ---- build bilateral kernel tiles: Kb[j, i] = exp(Fj.Fi - .5|Fj|^2 - .5|Fi|^2)
            with (
                tc.tile_pool(name="bpsum", bufs=2, space="PSUM") as bpsum,
                tc.tile_pool(name="atpool", bufs=2) as atpool,
                tc.tile_pool(name="spillst", bufs=2) as spillst,
            ):
                at_chunk = None
                for jt in range(TJ):
                    if jt % TB == 0:
                        at_chunk = atpool.tile([7, TB * 128], f32, tag="at")
                        nc.sync.dma_start(
                            at_chunk[:], at_d[:, jt * 128 : (jt + TB) * 128]
                        )
                    lhs = at_chunk[:, (jt % TB) * 128 : (jt % TB) * 128 + 128]
                    xp = bpsum.tile([128, 2048], f32, tag="xp")
                    for off, ln in CHUNKS:
                        nc.tensor.matmul(
                            xp[:, off : off + ln], lhs, bt[:, off : off + ln],
                            start=True, stop=True,
                        )
                    if jt < RES:
                        dest = kb_res[:, jt * BAND : (jt + 1) * BAND]
                        nc.scalar.activation(
                            dest, xp[:, :BAND], mybir.ActivationFunctionType.Exp
                        )
                    else:
                        stg = spillst.tile([128, BAND], bf16, tag="stg")
                        nc.scalar.activation(
                            stg[:], xp[:, :BAND], mybir.ActivationFunctionType.Exp
                        )
                        nc.sync.dma_start(
                            kb_spill[(jt - RES) * 128 : (jt - RES + 1) * 128, :],
                            stg[:],
                        )

            # ---- iterations ----
            with (
                tc.tile_pool(name="accs", bufs=1, space="PSUM") as accs_pool,
                tc.tile_pool(name="accb", bufs=1, space="PSUM") as accb_pool,
                tc.tile_pool(name="pwp", bufs=1, space="PSUM") as pw_pool,
            ):
                for it in range(NITER):
                    # softmax over channels (no max-subtract; |q| stays small)
                    q_all = state.tile([128, TJ * CP], f32, tag="qall")
                    if it == 0:
                        nc.sync.dma_start(q_all[:], uarr_d[:])
                    else:
                        nc.sync.dma_start(
                            q_all.rearrange("p (r f) -> p r f", r=NCORES)[:],
                            ag_out.rearrange(
                                "(r p f) -> p r f", r=NCORES, p=128
                            )[:],
                        )
                    e_all = state.tile([128, TJ * CP], f32, tag="eall")
                    nc.scalar.activation(
                        e_all[:], q_all[:], mybir.ActivationFunctionType.Exp
                    )
                    sums = small.tile([128, TJ], f32, tag="sums")
                    nc.vector.tensor_reduce(
                        sums[:],
                        e_all.rearrange("p (t c) -> p t c", c=CP)[:],
                        axis=mybir.AxisListType.X,
                        op=mybir.AluOpType.add,
                    )
                    rcp = small.tile([128, TJ], f32, tag="rcp")
                    nc.vector.reciprocal(rcp[:], sums[:])
                    p_all = state.tile([128, TJ * CP], bf16, tag="pall")
                    nc.vector.tensor_tensor(
                        p_all.rearrange("p (t c) -> p t c", c=CP)[:],
                        e_all.rearrange("p (t c) -> p t c", c=CP)[:],
                        rcp[:].unsqueeze(2).to_broadcast((128, TJ, CP)),
                        op=mybir.AluOpType.mult,
                    )

                    # main accumulation: S.T / Bi.T [CP, BAND]
                    acc_s = accs_pool.tile([CP, BAND], f32, tag="accs")
                    acc_b = accb_pool.tile([CP, BAND], f32, tag="accb")
                    ks_grp = None
                    kb_grp = None
                    for jt in range(TJ):
                        if jt % GRP == 0:
                            ks_grp = stream_pool.tile([128, GRP * BAND], bf16, tag="sg")
                            nc.sync.dma_start(
                                ks_grp.rearrange("p (g i) -> p g i", g=GRP)[:],
                                kst_d[jt * 128 : (jt + GRP) * 128, :].rearrange(
                                    "(g p) i -> p g i", p=128
                                ),
                            )
                        if jt >= RES and (jt - RES) % GRP == 0:
                            kb_grp = stream_pool.tile([128, GRP * BAND], bf16, tag="sg")
                            nc.sync.dma_start(
                                kb_grp.rearrange("p (g i) -> p g i", g=GRP)[:],
                                kb_spill[
                                    (jt - RES) * 128 : (jt - RES + GRP) * 128, :
                                ].rearrange("(g p) i -> p g i", p=128),
                            )
                        lhs = p_all[:, jt * CP : (jt + 1) * CP]
                        ks_base = (jt % GRP) * BAND
                        first, last = jt == 0, jt == TJ - 1
                        for off, ln in CHUNKS:
                            nc.tensor.matmul(
                                acc_s[:, off : off + ln], lhs,
                                ks_grp[:, ks_base + off : ks_base + off + ln],
                                start=first, stop=last,
                            )
                            if jt < RES:
                                kb_rhs = kb_res[:, jt * BAND + off : jt * BAND + off + ln]
                            else:
                                b = ((jt - RES) % GRP) * BAND
                                kb_rhs = kb_grp[:, b + off : b + off + ln]
                            nc.tensor.matmul(
                                acc_b[:, off : off + ln], lhs, kb_rhs,
                                start=first, stop=last,
                            )

                    # pairwise = S @ CsF + Bi @ CbF   (A-layout [128, TB*CP])
                    st_sb = state.tile([CP, BAND], f32, tag="stsb")
                    bit_sb = state.tile([CP, BAND], f32, tag="bitsb")
                    nc.scalar.copy(st_sb[:], acc_s[:])
                    nc.vector.tensor_copy(bit_sb[:], acc_b[:])
                    pw = pw_pool.tile([128, TB * CP], f32, tag="pw")
                    for ic in range(TB):
                        nc.tensor.matmul(
                            pw[:, ic * CP : (ic + 1) * CP],
                            st_sb[:, ic * 128 : (ic + 1) * 128],
                            csf[:], start=True, stop=False,
                        )
                        nc.tensor.matmul(
                            pw[:, ic * CP : (ic + 1) * CP],
                            bit_sb[:, ic * 128 : (ic + 1) * 128],
                            cbf[:], start=False, stop=True,
                        )

                    qnew = small.tile([128, TB * CP], f32, tag="qnew")
                    nc.vector.tensor_tensor(
                        qnew[:], u_band[:], pw[:], op=mybir.AluOpType.subtract
                    )

                    if it < NITER - 1:
                        ag_in = dram.tile([128 * TB * CP], f32, tag=f"agin{it}")
                        ag_out = dram.tile(
                            [NCORES * 128 * TB * CP], f32,
                            addr_space="Shared", tag=f"agout{it}",
                        )
                        nc.sync.dma_start(
                            ag_in.rearrange("(p f) -> p f", p=128)[:], qnew[:]
                        )
                        nc.gpsimd.collective_compute(
                            "AllGather",
                            mybir.AluOpType.bypass,
                            ins=[ag_in[:]],
                            outs=[ag_out[:]],
                            replica_groups=[list(range(NCORES))],
                        )
                    else:
                        nc.sync.dma_start(out_d[:], qnew[:])

    nc.compile()
    return nc


def _host_inputs(unaries, reference_image, spatial_ker_weights,
                 bilateral_ker_weights, compatibility_matrix):
    """Per-core input maps (all host work is O(N*D) layout prep + the
    position-only spatial kernel constant)."""
    u = np.asarray(unaries, np.float32).reshape(N, C)
    img = np.asarray(reference_image, np.float32).reshape(N, 3)
    ws = np.asarray(spatial_ker_weights, np.float32)
    wb = np.asarray(bilateral_ker_weights, np.float32)
    comp = np.asarray(compatibility_matrix, np.float32)

    yy, xx = np.meshgrid(np.arange(H, dtype=np.float32),
                         np.arange(W, dtype=np.float32), indexing="ij")
    pos = np.stack([yy.ravel(), xx.ravel()], axis=1)          # [N, 2]

    # bilateral augmented features: X[j,i] = F_j.F_i - .5|F_j|^2 - .5|F_i|^2
    F = np.concatenate([pos / THETA_ALPHA, img / THETA_BETA], axis=1)  # [N, 5]
    sq = 0.5 * (F * F).sum(axis=1)                                     # [N]
    ones = np.ones((N, 1), np.float32)
    at = np.concatenate([F, ones, -sq[:, None]], axis=1).T.astype(np.float32)  # [7, N]
    bt_full = np.concatenate([F, -sq[:, None], ones], axis=1).T.astype(np.float32)

    # padded u, folded compat
    u_pad = np.full((N, CP), NEG, np.float32)
    u_pad[:, :C] = u
    csf = np.zeros((CP, CP), np.float32)
    cbf = np.zeros((CP, CP), np.float32)
    csf[:C, :C] = ws[:, None] * comp.T          # CsF[c,k] = ws[c] * C[k,c]
    cbf[:C, :C] = wb[:, None] * comp.T

    # u in the partition-major allgather block layout
    uarr = u_pad.reshape(TJ, 128, CP).transpose(1, 0, 2).reshape(128, TJ * CP)

    # spatial kernel: position-only constant
    py, px = pos[:, 0], pos[:, 1]
    in_maps = []
    for r in range(NCORES):
        band = slice(r * BAND, (r + 1) * BAND)
        d2 = (py[:, None] - py[None, band]) ** 2 + (px[:, None] - px[None, band]) ** 2
        kst = np.exp(d2 * (-0.5 / (THETA_GAMMA * THETA_GAMMA))).astype(
            ml_dtypes.bfloat16
        )                                                       # [N, BAND]
        uband = (
            u_pad[band]
            .reshape(TB, 128, CP)
            .transpose(1, 0, 2)
            .reshape(128, TB * CP)
        )
        in_maps.append({
            "at": at,
            "bt": np.ascontiguousarray(bt_full[:, band]),
            "kst": kst,
            "uarr": uarr,
            "uband": uband,
            "csf": csf,
            "cbf": cbf,
        })
    return in_maps


def _run(in_maps, trace=False, **kw):
    if "nc" not in _CACHE:
        _CACHE["nc"] = _build_nc()
    return run_bass_kernel_spmd(
        _CACHE["nc"], in_maps, list(range(NCORES)), trace=trace, **kw
    )


def _assemble(results):
    bands = []
    for r in range(NCORES):
        arr = results[r]["out"]                              # [128, TB*CP]
        band = arr.reshape(128, TB, CP).transpose(1, 0, 2).reshape(BAND, CP)
        bands.append(band[:, :C])
    return np.concatenate(bands, axis=0).reshape(1, H, W, C).astype(np.float32)


def kernel(unaries, reference_image, spatial_ker_weights,
           bilateral_ker_weights, compatibility_matrix):
    in_maps = _host_inputs(
        unaries, reference_image, spatial_ker_weights,
        bilateral_ker_weights, compatibility_matrix,
    )
    res = _run(in_maps, trace=False)
    return _assemble(res.results)


# revision 10
# speedup vs baseline: 1.0683x; 1.0683x over previous
"""CRF-RNN layer (nn_CRF_RNN_Layer) as a Bass/Tile kernel on 8 trn2 NeuronCores.

Math (reference):
    N = 96*96 pixels, C = 21 classes, 5 mean-field iterations.
    k_spatial / k_bilateral are [N, N] Gaussian kernels; per iteration:
        p = softmax(q); S = Ks @ p; Bi = Kb @ p
        pairwise = (S * ws + Bi * wb) @ C.T;  q = u - pairwise

Device strategy:
    - Row-shard outputs over 8 cores (1152 rows each).
    - Channels padded 21 -> 32; pad logits are -1e30 so softmax pads are 0.
    - Per-channel weights + compat fold on host: CsF[c,k] = ws[c]*C[k,c].
    - Big matmuls in out.T form: S.T[c, i] = sum_j p[j, c] * K[j, i],
      with lhsT = p tiles [128, 32] bf16 and rhs = K tiles [128, 1152] bf16.
    - Kb (data-dependent) is built on device: X = Faug_j . Gaug_i via a
      7-contraction matmul, exp on the scalar engine, stored bf16 (mostly
      SBUF-resident; every 4th tile spilled to HBM and re-streamed).
    - Ks depends only on pixel positions -> host constant, streamed bf16.
    - Each core softmaxes its own band and the cores AllGather the bf16
      probabilities (74KB/rank) in a partition-major block layout so all
      DMA runs are contiguous.
"""

import numpy as np
import ml_dtypes

from concourse import bacc, mybir, tile
from concourse.bass_utils import run_bass_kernel_spmd

H, W, C = 96, 96, 21
THETA_ALPHA, THETA_BETA, THETA_GAMMA = 8.0, 0.125, 3.0
NITER = 5
NCORES = 8
N = H * W                     # 9216
BAND = N // NCORES            # 1152 rows per core
CP = 32                       # padded channels
TJ = N // 128                 # 72 j-tiles
TB = BAND // 128              # 9 band tiles
SPILL_MOD = 4                 # j-tiles with jt % 4 == 2 are spilled to HBM
GRP = 6                       # Ks j-tiles per streaming DMA (1.77 MB)
SGRP = 6                 # spilled-Kb j-tiles per streaming DMA
NEG = -1.0e30
CHUNKS = [(0, 512), (512, 512), (1024, 128)]   # psum-bank-aligned N splits of 1152

SPILLED = [jt for jt in range(TJ) if jt % SPILL_MOD == 2] + [69, 71]
S_IDX = {jt: s for s, jt in enumerate(SPILLED)}
NSPILL = len(SPILLED)
RES_IDX = {}
for jt in range(TJ):
    if jt not in S_IDX:
        RES_IDX[jt] = len(RES_IDX)
NRES = len(RES_IDX)

_CACHE = {}


def _build_nc():
    nc = bacc.Bacc("TRN2", target_bir_lowering=False, debug=False, num_devices=NCORES)
    f32 = mybir.dt.float32
    bf16 = mybir.dt.bfloat16

    at_d = nc.declare_dram_parameter("at", [7, N], f32, isOutput=False)
    bt_d = nc.declare_dram_parameter("bt", [7, BAND], f32, isOutput=False)
    kst_d = nc.declare_dram_parameter("kst", [N, BAND], bf16, isOutput=False)
    uband_d = nc.declare_dram_parameter("uband", [128, TB * CP], f32, isOutput=False)
    csf_d = nc.declare_dram_parameter("csf", [CP, CP], f32, isOutput=False)
    cbf_d = nc.declare_dram_parameter("cbf", [CP, CP], f32, isOutput=False)
    out_d = nc.declare_dram_parameter("out", [128, TB * CP], f32, isOutput=True)

    with tile.TileContext(nc) as tc:
        with (
            tc.tile_pool(name="kres", bufs=1) as kres_pool,
            tc.tile_pool(name="ksstream", bufs=2) as ks_pool,
            tc.tile_pool(name="spstream", bufs=2) as sp_pool,
            tc.tile_pool(name="state", bufs=1) as state,
            tc.tile_pool(name="small", bufs=2) as small,
            tc.tile_pool(name="dram", bufs=1, space="DRAM") as dram,
        ):
            # ---- constants ----
            csf = state.tile([CP, CP], f32, tag="csf")
            cbf = state.tile([CP, CP], f32, tag="cbf")
            u_band = state.tile([128, TB * CP], f32, tag="uband")
            bt = state.tile([7, BAND], f32, tag="bt")
            nc.sync.dma_start(csf[:], csf_d[:])
            nc.sync.dma_start(cbf[:], cbf_d[:])
            nc.sync.dma_start(u_band[:], uband_d[:])
            nc.sync.dma_start(bt[:], bt_d[:])

            kb_res = kres_pool.tile([128, NRES * BAND], bf16, tag="kbres")
            kb_spill = dram.tile([NSPILL * 128, BAND], bf16)

            # ---- iteration-0: band softmax of u + AllGather (overlaps build) ----
            def band_softmax_ag(src_tile, it):
                eb = small.tile([128, TB * CP], f32, tag="eb")
                nc.scalar.activation(
                    eb[:], src_tile[:], mybir.ActivationFunctionType.Exp
                )
                sb = small.tile([128, TB], f32, tag="sb")
                nc.vector.tensor_reduce(
                    sb[:],
                    eb.rearrange("p (t c) -> p t c", c=CP)[:],
                    axis=mybir.AxisListType.X,
                    op=mybir.AluOpType.add,
                )
                rb = small.tile([128, TB], f32, tag="rb")
                nc.vector.reciprocal(rb[:], sb[:])
                pband = small.tile([128, TB * CP], bf16, tag="pband")
                nc.vector.tensor_tensor(
                    pband.rearrange("p (t c) -> p t c", c=CP)[:],
                    eb.rearrange("p (t c) -> p t c", c=CP)[:],
                    rb[:].unsqueeze(2).to_broadcast((128, TB, CP)),
                    op=mybir.AluOpType.mult,
                )
                ag_in = dram.tile([128 * TB * CP], bf16, tag=f"agin{it}")
                ag_out = dram.tile(
                    [NCORES * 128 * TB * CP], bf16,
                    addr_space="Shared", tag=f"agout{it}",
                )
                nc.sync.dma_start(
                    ag_in.rearrange("(p f) -> p f", p=128)[:], pband[:]
                )
                nc.gpsimd.collective_compute(
                    "AllGather",
                    mybir.AluOpType.bypass,
                    ins=[ag_in[:]],
                    outs=[ag_out[:]],
                    replica_groups=[list(range(NCORES))],
                )
                return ag_out

            ag_out = band_softmax_ag(u_band, 0)

            # ---- build bilateral kernel: Kb[j, i] = exp(Fj.Fi - .5|Fj|^2 - .5|Fi|^2)
            with (
                tc.tile_pool(name="bpsum", bufs=2, space="PSUM") as bpsum,
                tc.tile_pool(name="atpool", bufs=2) as atpool,
                tc.tile_pool(name="spillst", bufs=2) as spillst,
            ):
                at_chunk = None
                for jt in range(TJ):
                    if jt % 3 == 0:
                        at_chunk = atpool.tile([7, 3 * 128], f32, tag="at")
                        nc.sync.dma_start(
                            at_chunk[:], at_d[:, jt * 128 : (jt + 3) * 128]
                        )
                    lhs = at_chunk[:, (jt % 3) * 128 : (jt % 3) * 128 + 128]
                    xp = bpsum.tile([128, BAND], f32, tag="xp")
                    for off, ln in CHUNKS:
                        nc.tensor.matmul(
                            xp[:, off : off + ln], lhs, bt[:, off : off + ln],
                            start=True, stop=True,
                        )
                    if jt in RES_IDX:
                        dest = kb_res[
                            :, RES_IDX[jt] * BAND : (RES_IDX[jt] + 1) * BAND
                        ]
                        nc.scalar.activation(
                            dest, xp[:], mybir.ActivationFunctionType.Exp
                        )
                    else:
                        stg = spillst.tile([128, BAND], bf16, tag="stg")
                        nc.scalar.activation(
                            stg[:], xp[:], mybir.ActivationFunctionType.Exp
                        )
                        s = S_IDX[jt]
                        nc.sync.dma_start(
                            kb_spill[s * 128 : (s + 1) * 128, :], stg[:]
                        )

            # ---- iterations ----
            with (
                tc.tile_pool(name="accs", bufs=1, space="PSUM") as accs_pool,
                tc.tile_pool(name="accb", bufs=1, space="PSUM") as accb_pool,
                tc.tile_pool(name="pwp", bufs=1, space="PSUM") as pw_pool,
            ):
                for it in range(NITER):
                    p_all = state.tile([128, TJ * CP], bf16, tag="pall")
                    nc.sync.dma_start(
                        p_all.rearrange("p (r f) -> p r f", r=NCORES)[:],
                        ag_out.rearrange(
                            "(r p f) -> p r f", r=NCORES, p=128
                        )[:],
                    )

                    # main accumulation: S.T / Bi.T [CP, BAND]
                    acc_s = accs_pool.tile([CP, BAND], f32, tag="accs")
                    acc_b = accb_pool.tile([CP, BAND], f32, tag="accb")
                    ks_grp = None
                    sp_grp = None
                    for jt in range(TJ):
                        if jt % GRP == 0:
                            ks_grp = ks_pool.tile([128, GRP * BAND], bf16, tag="ks")
                            nc.sync.dma_start(
                                ks_grp.rearrange("p (g i) -> p g i", g=GRP)[:],
                                kst_d[jt * 128 : (jt + GRP) * 128, :].rearrange(
                                    "(g p) i -> p g i", p=128
                                ),
                            )
                        if jt in S_IDX and S_IDX[jt] % SGRP == 0:
                            s0 = S_IDX[jt]
                            ng = min(SGRP, NSPILL - s0)
                            sp_grp = sp_pool.tile(
                                [128, SGRP * BAND], bf16, tag="sp"
                            )
                            nc.sync.dma_start(
                                sp_grp.rearrange("p (g i) -> p g i", g=SGRP)[
                                    :, :ng, :
                                ],
                                kb_spill[
                                    s0 * 128 : (s0 + ng) * 128, :
                                ].rearrange("(g p) i -> p g i", p=128),
                            )
                        lhs = p_all[:, jt * CP : (jt + 1) * CP]
                        ksb = (jt % GRP) * BAND
                        if jt in RES_IDX:
                            kb_base = RES_IDX[jt] * BAND
                            kb_src = kb_res
                        else:
                            kb_base = (S_IDX[jt] % SGRP) * BAND
                            kb_src = sp_grp
                        first, last = jt == 0, jt == TJ - 1
                        for off, ln in CHUNKS:
                            nc.tensor.matmul(
                                acc_s[:, off : off + ln], lhs,
                                ks_grp[:, ksb + off : ksb + off + ln],
                                start=first, stop=last,
                            )
                            nc.tensor.matmul(
                                acc_b[:, off : off + ln], lhs,
                                kb_src[:, kb_base + off : kb_base + off + ln],
                                start=first, stop=last,
                            )

                    # pairwise = S @ CsF + Bi @ CbF   (A-layout [128, TB*CP])
                    st_sb = state.tile([CP, BAND], f32, tag="stsb")
                    bit_sb = state.tile([CP, BAND], f32, tag="bitsb")
                    nc.scalar.copy(st_sb[:], acc_s[:])
                    nc.vector.tensor_copy(bit_sb[:], acc_b[:])
                    pw = pw_pool.tile([128, TB * CP], f32, tag="pw")
                    for ic in range(TB):
                        nc.tensor.matmul(
                            pw[:, ic * CP : (ic + 1) * CP],
                            st_sb[:, ic * 128 : (ic + 1) * 128],
                            csf[:], start=True, stop=False,
                        )
                        nc.tensor.matmul(
                            pw[:, ic * CP : (ic + 1) * CP],
                            bit_sb[:, ic * 128 : (ic + 1) * 128],
                            cbf[:], start=False, stop=True,
                        )

                    qnew = small.tile([128, TB * CP], f32, tag="qnew")
                    nc.vector.tensor_tensor(
                        qnew[:], u_band[:], pw[:], op=mybir.AluOpType.subtract
                    )

                    if it < NITER - 1:
                        ag_out = band_softmax_ag(qnew, it + 1)
                    else:
                        nc.sync.dma_start(out_d[:], qnew[:])

    nc.compile()
    return nc


def _host_inputs(unaries, reference_image, spatial_ker_weights,
                 bilateral_ker_weights, compatibility_matrix):
    """Per-core input maps (all host work is O(N*D) layout prep + the
    position-only spatial kernel constant)."""
    u = np.asarray(unaries, np.float32).reshape(N, C)
    img = np.asarray(reference_image, np.float32).reshape(N, 3)
    ws = np.asarray(spatial_ker_weights, np.float32)
    wb = np.asarray(bilateral_ker_weights, np.float32)
    comp = np.asarray(compatibility_matrix, np.float32)

    yy, xx = np.meshgrid(np.arange(H, dtype=np.float32),
                         np.arange(W, dtype=np.float32), indexing="ij")
    pos = np.stack([yy.ravel(), xx.ravel()], axis=1)          # [N, 2]

    # bilateral augmented features: X[j,i] = F_j.F_i - .5|F_j|^2 - .5|F_i|^2
    F = np.concatenate([pos / THETA_ALPHA, img / THETA_BETA], axis=1)  # [N, 5]
    sq = 0.5 * (F * F).sum(axis=1)                                     # [N]
    ones = np.ones((N, 1), np.float32)
    at = np.concatenate([F, ones, -sq[:, None]], axis=1).T.astype(np.float32)  # [7, N]
    bt_full = np.concatenate([F, -sq[:, None], ones], axis=1).T.astype(np.float32)

    # padded u, folded compat
    u_pad = np.full((N, CP), NEG, np.float32)
    u_pad[:, :C] = u
    csf = np.zeros((CP, CP), np.float32)
    cbf = np.zeros((CP, CP), np.float32)
    csf[:C, :C] = ws[:, None] * comp.T          # CsF[c,k] = ws[c] * C[k,c]
    cbf[:C, :C] = wb[:, None] * comp.T

    # spatial kernel: position-only constant
    py, px = pos[:, 0], pos[:, 1]
    in_maps = []
    for r in range(NCORES):
        band = slice(r * BAND, (r + 1) * BAND)
        d2 = (py[:, None] - py[None, band]) ** 2 + (px[:, None] - px[None, band]) ** 2
        kst = np.exp(d2 * (-0.5 / (THETA_GAMMA * THETA_GAMMA))).astype(
            ml_dtypes.bfloat16
        )                                                       # [N, BAND]
        uband = (
            u_pad[band]
            .reshape(TB, 128, CP)
            .transpose(1, 0, 2)
            .reshape(128, TB * CP)
        )
        in_maps.append({
            "at": at,
            "bt": np.ascontiguousarray(bt_full[:, band]),
            "kst": kst,
            "uband": uband,
            "csf": csf,
            "cbf": cbf,
        })
    return in_maps


def _run(in_maps, trace=False, **kw):
    if "nc" not in _CACHE:
        _CACHE["nc"] = _build_nc()
    return run_bass_kernel_spmd(
        _CACHE["nc"], in_maps, list(range(NCORES)), trace=trace, **kw
    )


def _assemble(results):
    bands = []
    for r in range(NCORES):
        arr = results[r]["out"]                              # [128, TB*CP]
        band = arr.reshape(128, TB, CP).transpose(1, 0, 2).reshape(BAND, CP)
        bands.append(band[:, :C])
    return np.concatenate(bands, axis=0).reshape(1, H, W, C).astype(np.float32)


def kernel(unaries, reference_image, spatial_ker_weights,
           bilateral_ker_weights, compatibility_matrix):
    in_maps = _host_inputs(
        unaries, reference_image, spatial_ker_weights,
        bilateral_ker_weights, compatibility_matrix,
    )
    res = _run(in_maps, trace=False)
    return _assemble(res.results)
